# revision 1
# baseline (speedup 1.0000x reference)
"""RIENet loss kernel (keypoint/KNN MSE + global-align Huber-min loss) on 8 trn2 cores.

Sharding: core ci -> (b = ci // 4, n-chunk j = ci % 4).  Each core holds the full
tgt[b] (M=8192 points) and a 2048-column chunk of src_transformed[b] (N axis).
  loss_1 (min over M per src point): complete locally per core.
  loss_2 (min over N per tgt point): per-core partial min over its chunk;
          host min-reduces the 4 chunks per batch element.

Device kernel per core (v2 — bf16-split matmul, PE off the critical path):
  Q[m, n] = -2 t_m . s_n + ||s_n||^2 computed by one K=21 bf16 matmul:
  t and s are split 3-way into bf16 (hi/mid/lo, ~27 mantissa bits total) and
  the 6 dominant cross products are taken (error ~1e-6 absolute); ||s||^2 is
  split 3-way against ones-rows.  ||t_m||^2 stays fp32 and is folded in
  per-partition by scalar_tensor_tensor during the column-min accumulation:
    acc = min(Q + nt[m], acc)           (min over m-tiles, DVE, one pass)
    rowbuf[:, mi] = reduce_min(Q)       (min over n-chunk, DVE, one pass)
  rowbuf gets nt added at the end; acc is partition-min-reduced via PE
  transposes.  Tiny keypoint/KNN MSE losses run on-device on every core.
"""

import os
import numpy as np


def _ensure_path():
    try:
        import concourse  # noqa: F401
    except ImportError:
        import sys
        for p in ("/opt/trn_rl_repo", "/root/.axon_site/_ro/trn_rl_repo"):
            if os.path.isdir(p) and p not in sys.path:
                sys.path.insert(0, p)


_ensure_path()

import concourse.bass as bass  # noqa: E402
import concourse.bacc as bacc  # noqa: E402
import concourse.tile as tile  # noqa: E402
import concourse.mybir as mybir  # noqa: E402
from concourse.bass_utils import run_bass_kernel_spmd  # noqa: E402

F32 = mybir.dt.float32
BF16 = mybir.dt.bfloat16
AL = mybir.AluOpType
AF = mybir.ActivationFunctionType

MARGIN = 0.1
B, KP, KNN, N, M = 2, 256, 32, 8192, 8192
NCORES = 8
NSHARDS = NCORES // B          # 4 n-chunks per batch element
CHUNK = N // NSHARDS           # 2048
NJ = CHUNK // 512              # 4 psum banks per m-tile
MI = M // 128                  # 64 m-tiles
GT = M // 128                  # 64 groups in the [p, d, g] tgt layout
GS = CHUNK // 128              # 16 groups in the [p, d, g] src layout
K21 = 21
BIG = 3.0e38

_CACHE = {}


def _build():
    nc = bacc.Bacc("TRN2", target_bir_lowering=False, debug=False,
                   num_devices=NCORES)

    src = nc.dram_tensor("src", [3, CHUNK], F32, kind="ExternalInput")
    tgt = nc.dram_tensor("tgt", [3, M], F32, kind="ExternalInput")
    ident = nc.dram_tensor("ident", [128, 128], F32, kind="ExternalInput")
    kp_lhsT = nc.dram_tensor("kp_lhsT", [4, 2 * 3], F32, kind="ExternalInput")
    kp_rhs = nc.dram_tensor("kp_rhs", [4, 2 * KP], F32, kind="ExternalInput")
    tgt_kp = nc.dram_tensor("tgt_kp", [3, 2 * KP], F32, kind="ExternalInput")
    knn_src = nc.dram_tensor("knn_src", [128, 2 * 192], F32, kind="ExternalInput")
    knn_tgt = nc.dram_tensor("knn_tgt", [128, 2 * 192], F32, kind="ExternalInput")

    colmin_o = nc.dram_tensor("colmin", [128, CHUNK // 128], F32, kind="ExternalOutput")
    rowmin_o = nc.dram_tensor("rowmin", [128, MI], F32, kind="ExternalOutput")
    misc_o = nc.dram_tensor("misc", [128, 4], F32, kind="ExternalOutput")

    with tile.TileContext(nc) as tc:
        with (
            tc.tile_pool(name="const", bufs=1) as const,
            tc.tile_pool(name="sc", bufs=3) as sc,
        ):
            tA = const.tile([K21, M], BF16)       # lhsT rows
            sA = const.tile([K21, CHUNK], BF16)   # rhs rows
            acc = const.tile([128, CHUNK], F32)
            rowbuf = const.tile([128, MI], F32)
            nt_all = const.tile([128, GT], F32)   # ||t||^2, [p, mi]
            id_sb = const.tile([128, 128], F32)
            colmin_sb = const.tile([128, CHUNK // 128], F32)
            misc_sb = const.tile([128, 4], F32)

            nc.sync.dma_start(out=id_sb[:], in_=ident[:])
            nc.gpsimd.memset(acc[:], BIG)
            nc.gpsimd.memset(misc_sb[:], 0.0)

            # ---- load t, s in [p, d, g] layouts (partition-minor DMA) ----
            tw = const.tile([128, 3, GT], F32)
            sw = const.tile([128, 3, GS], F32)
            nc.sync.dma_start(out=tw[:], in_=tgt.rearrange("d (g p) -> p d g", p=128))
            nc.sync.dma_start(out=sw[:], in_=src.rearrange("d (g p) -> p d g", p=128))

            # ---- norms (fp32) ----
            tsq = const.tile([128, 3, GT], F32)
            nc.vector.tensor_mul(tsq[:], tw[:], tw[:])
            nc.vector.tensor_add(nt_all[:], tsq[:, 0, :], tsq[:, 1, :])
            nc.vector.tensor_add(nt_all[:], nt_all[:], tsq[:, 2, :])
            ssq = const.tile([128, 3, GS], F32)
            ns_w = const.tile([128, GS], F32)
            nc.vector.tensor_mul(ssq[:], sw[:], sw[:])
            nc.vector.tensor_add(ns_w[:], ssq[:, 0, :], ssq[:, 1, :])
            nc.vector.tensor_add(ns_w[:], ns_w[:], ssq[:, 2, :])

            # ---- 3-way bf16 splits (kept as exactly-rounded fp32 tiles) ----
            nc.scalar.mul(out=tw[:], in_=tw[:], mul=-2.0)  # fold -2 into t side

            def split3(name, w, shape):
                outs = []
                cur = w
                for lvl in range(3):
                    b16 = sc.tile(shape, BF16, tag=f"{name}_b{lvl}")
                    nc.scalar.copy(out=b16[:], in_=cur[:])
                    f32t = const.tile(shape, F32, tag=f"{name}_f{lvl}")
                    nc.vector.tensor_copy(out=f32t[:], in_=b16[:])
                    outs.append(f32t)
                    if lvl < 2:
                        nxt = const.tile(shape, F32, tag=f"{name}_r{lvl}")
                        nc.vector.tensor_sub(nxt[:], cur[:], f32t[:])
                        cur = nxt
                return outs

            th, tm, tl = split3("t", tw, [128, 3, GT])
            sh, sm, sl = split3("s", sw, [128, 3, GS])
            nsp = split3("n", ns_w.rearrange("p (o g) -> p o g", o=1),
                         [128, 1, GS])

            # pairing layout: lhsT rows [th,th,tm,tm,th,tl]*3d + ones*3
            #                 rhs  rows [sh,sm,sh,sm,sl,sh]*3d + ns_splits
            t_dest = {0: [0, 3, 12], 1: [6, 9], 2: [15]}    # th, tm, tl
            s_dest = {0: [0, 6, 15], 1: [3, 9], 2: [12]}    # sh, sm, sl

            with tc.tile_pool(name="psum_pre", bufs=4, space="PSUM") as pp:
                def place(w_f32, groups, dst_tile, rows, width):
                    # transpose [128, g] -> [g, 128] via PE, cast to bf16,
                    # then DMA into row(s) of the operand tile
                    pt = pp.tile([groups, 128], F32, tag=f"tp{groups}")
                    nc.tensor.transpose(pt[:], w_f32, id_sb[:])
                    tr = sc.tile([groups, 128], BF16, tag=f"tr{groups}")
                    nc.scalar.copy(out=tr[:], in_=pt[:])
                    for r in rows:
                        nc.sync.dma_start(
                            out=dst_tile[r:r + 1, :].rearrange(
                                "o (g p) -> o g p", p=128),
                            in_=tr[:])

                for lvl, w in enumerate([th, tm, tl]):
                    for d in range(3):
                        place(w[:, d, :], GT, tA,
                              [base + d for base in t_dest[lvl]], M)
                for lvl, w in enumerate([sh, sm, sl]):
                    for d in range(3):
                        place(w[:, d, :], GS, sA,
                              [base + d for base in s_dest[lvl]], CHUNK)
                for lvl in range(3):
                    place(nsp[lvl][:, 0, :], GS, sA, [18 + lvl], CHUNK)

                # ones rows 18-20 of lhsT (staged at partition 0, DMA'd up)
                ones3 = const.tile([3, M], BF16)
                nc.vector.memset(ones3[:], 1.0)
                nc.sync.dma_start(out=tA[18:21, :], in_=ones3[:])

            # ---- main loop: Q = -2 t.s + ||s||^2 per 128-row m-tile ----
            with tc.tile_pool(name="psum_main", bufs=2, space="PSUM") as pm:
                for mi in range(MI):
                    pt = pm.tile([128, CHUNK], F32, tag="pt")
                    for nj in range(NJ):
                        nc.tensor.matmul(
                            pt[:, nj * 512:(nj + 1) * 512],
                            lhsT=tA[:, mi * 128:(mi + 1) * 128],
                            rhs=sA[:, nj * 512:(nj + 1) * 512],
                            start=True, stop=True,
                        )
                    # colmin: acc = min(Q + nt[m], acc)
                    nc.vector.scalar_tensor_tensor(
                        out=acc[:], in0=pt[:], scalar=nt_all[:, mi:mi + 1],
                        in1=acc[:], op0=AL.add, op1=AL.min)
                    # rowmin over the n-chunk (nt added after the loop)
                    nc.vector.tensor_reduce(
                        out=rowbuf[:, mi:mi + 1], in_=pt[:],
                        axis=mybir.AxisListType.X, op=AL.min)

            nc.vector.tensor_add(rowbuf[:], rowbuf[:], nt_all[:])

            with tc.tile_pool(name="psum_fin", bufs=2, space="PSUM") as pf:
                # partition-axis min of acc via PE transposes
                for blk in range(CHUNK // 128):
                    tp = pf.tile([128, 128], F32, tag="tp")
                    nc.tensor.transpose(tp[:], acc[:, blk * 128:(blk + 1) * 128],
                                        id_sb[:])
                    nc.vector.tensor_reduce(
                        out=colmin_sb[:, blk:blk + 1], in_=tp[:],
                        axis=mybir.AxisListType.X, op=AL.min)

                # tiny keypoint / knn losses (both batch elements)
                kp_l = const.tile([4, 2 * 3], F32)
                kp_r = const.tile([4, 2 * KP], F32)
                kp_t = const.tile([3, 2 * KP], F32)
                ks = const.tile([128, 2 * 192], F32)
                kt = const.tile([128, 2 * 192], F32)
                nc.sync.dma_start(out=kp_l[:], in_=kp_lhsT[:])
                nc.sync.dma_start(out=kp_r[:], in_=kp_rhs[:])
                nc.sync.dma_start(out=kp_t[:], in_=tgt_kp[:])
                nc.sync.dma_start(out=ks[:], in_=knn_src[:])
                nc.sync.dma_start(out=kt[:], in_=knn_tgt[:])
                for b in range(B):
                    pt2 = pf.tile([3, KP], F32, tag="kp")
                    nc.tensor.matmul(
                        pt2[:], lhsT=kp_l[:, b * 3:(b + 1) * 3],
                        rhs=kp_r[:, b * KP:(b + 1) * KP],
                        start=True, stop=True)
                    diff = sc.tile([3, KP], F32, tag="kdiff")
                    nc.vector.tensor_sub(diff[:], pt2[:],
                                         kp_t[:, b * KP:(b + 1) * KP])
                    nc.vector.tensor_mul(diff[:], diff[:], diff[:])
                    nc.vector.tensor_reduce(
                        out=misc_sb[0:3, b:b + 1], in_=diff[:],
                        axis=mybir.AxisListType.X, op=AL.add)
                    diff2 = sc.tile([128, 192], F32, tag="ndiff")
                    nc.vector.tensor_sub(diff2[:], ks[:, b * 192:(b + 1) * 192],
                                         kt[:, b * 192:(b + 1) * 192])
                    nc.vector.tensor_mul(diff2[:], diff2[:], diff2[:])
                    nc.vector.tensor_reduce(
                        out=misc_sb[:, 2 + b:3 + b], in_=diff2[:],
                        axis=mybir.AxisListType.X, op=AL.add)

            nc.sync.dma_start(out=colmin_o[:], in_=colmin_sb[:])
            nc.sync.dma_start(out=rowmin_o[:], in_=rowbuf[:])
            nc.sync.dma_start(out=misc_o[:], in_=misc_sb[:])

    nc.compile()
    return nc


def _get_nc():
    if "nc" not in _CACHE:
        _CACHE["nc"] = _build()
    return _CACHE["nc"]


def _prepare_in_maps(src_keypoints, tgt_keypoints, rotation_ab, translation_ab,
                     src_keypoints_knn, tgt_keypoints_knn, src_transformed, tgt):
    f = np.float32
    st = np.ascontiguousarray(np.asarray(src_transformed, dtype=f))
    tg = np.ascontiguousarray(np.asarray(tgt, dtype=f))
    skp = np.asarray(src_keypoints, dtype=f)
    tkp = np.asarray(tgt_keypoints, dtype=f)
    rot = np.asarray(rotation_ab, dtype=f)
    tra = np.asarray(translation_ab, dtype=f)
    sknn = np.asarray(src_keypoints_knn, dtype=f)
    tknn = np.asarray(tgt_keypoints_knn, dtype=f)

    ident = np.eye(128, dtype=f)
    kp_lhsT = np.zeros((4, 2 * 3), dtype=f)
    kp_rhs = np.zeros((4, 2 * KP), dtype=f)
    tgt_kp = np.zeros((3, 2 * KP), dtype=f)
    knn_src = np.zeros((128, 2 * 192), dtype=f)
    knn_tgt = np.zeros((128, 2 * 192), dtype=f)
    for b in range(B):
        kp_lhsT[0:3, b * 3:(b + 1) * 3] = rot[b].T
        kp_lhsT[3, b * 3:(b + 1) * 3] = tra[b]
        kp_rhs[0:3, b * KP:(b + 1) * KP] = skp[b]
        kp_rhs[3, b * KP:(b + 1) * KP] = 1.0
        tgt_kp[:, b * KP:(b + 1) * KP] = tkp[b]
        knn_src[:, b * 192:(b + 1) * 192] = sknn[b].reshape(128, 192)
        knn_tgt[:, b * 192:(b + 1) * 192] = tknn[b].reshape(128, 192)

    shared = {
        "ident": ident, "kp_lhsT": kp_lhsT, "kp_rhs": kp_rhs,
        "tgt_kp": tgt_kp, "knn_src": knn_src, "knn_tgt": knn_tgt,
    }
    in_maps = []
    for ci in range(NCORES):
        b, j = divmod(ci, NSHARDS)
        m = dict(shared)
        m["src"] = np.ascontiguousarray(st[b, :, j * CHUNK:(j + 1) * CHUNK])
        m["tgt"] = tg[b]
        in_maps.append(m)
    return in_maps


def _huber(x, c):
    return np.where(x < c, 0.5 * x * x, c * x - 0.5 * c * c)


def _postprocess(results):
    c = np.float64(MARGIN)
    loss1 = np.float64(0.0)
    loss2 = np.float64(0.0)
    for b in range(B):
        rowmins = []
        for j in range(NSHARDS):
            r = results[b * NSHARDS + j]
            colmin = np.asarray(r["colmin"], dtype=np.float64).T.ravel()
            loss1 += _huber(colmin, c).sum()
            rowmins.append(np.asarray(r["rowmin"], dtype=np.float64).T.ravel())
        rm = np.minimum.reduce(rowmins)
        loss2 += _huber(rm, c).sum()
    gal = loss1 + loss2

    misc = np.asarray(results[0]["misc"], dtype=np.float64)
    kp_loss = (misc[0:3, 0].sum() + misc[0:3, 1].sum()) / B
    knn_loss = (misc[:, 2].sum() + misc[:, 3].sum()) / (B * KNN)
    ncl = knn_loss + kp_loss
    return np.float32(ncl), np.float32(gal)


def run_device(in_maps, **kw):
    nc = _get_nc()
    return run_bass_kernel_spmd(nc, in_maps, list(range(NCORES)), **kw)


def kernel(src_keypoints, tgt_keypoints, rotation_ab, translation_ab,
           src_keypoints_knn, tgt_keypoints_knn, k, src_transformed, tgt,
           **_unused):
    in_maps = _prepare_in_maps(src_keypoints, tgt_keypoints, rotation_ab,
                               translation_ab, src_keypoints_knn,
                               tgt_keypoints_knn, src_transformed, tgt)
    res = run_device(in_maps)
    return _postprocess(res.results)



# revision 2
# speedup vs baseline: 2.2457x; 2.2457x over previous
"""RIENet loss kernel (keypoint/KNN MSE + global-align Huber-min loss) on 8 trn2 cores.

Sharding: core ci -> (b = ci // 4, n-chunk j = ci % 4).  Each core holds the full
tgt[b] (M=8192 points) and a 2048-column chunk of src_transformed[b] (N axis).
  loss_1 (min over M per src point): per-core partial over the partition axis,
          finished on host (min over 128 partitions of the DMA'd acc tile).
  loss_2 (min over N per tgt point): per-core partial min over its chunk;
          host min-reduces the 4 chunks per batch element.

v3: all operand prep happens on the HOST (bf16 3-way splits, norms, row
layouts) so the device runs only the steady-state loop:
  PE     : Q[m-tile, :] = -2 t.s + ||s||^2 via K=21 bf16 matmul -> PSUM fp32
  ScalarE: Qb = bf16(Q + ||t_m||^2)   (activation Identity, per-partition bias)
  DVE    : acc  = min(acc, Qb)                     (TT bf16 SBUF, 2x mode)
           r1   = min(Qb[:, :1024], Qb[:, 1024:])  (TT 2x)
           r2   = min(r1[:, :512],  r1[:, 512:])   (TT 2x)
           rowbuf[:, mi] = reduce_min(r2)          (1x, 256 wide)
acc ([128, 2048] bf16, min over m per (partition, n) position) is DMA'd out
raw; the cross-partition min + Huber + sums happen on host (tiny).
Tiny keypoint/KNN MSE losses run on-device at the end of every core.
"""

import os
import numpy as np


def _ensure_path():
    try:
        import concourse  # noqa: F401
    except ImportError:
        import sys
        for p in ("/opt/trn_rl_repo", "/root/.axon_site/_ro/trn_rl_repo"):
            if os.path.isdir(p) and p not in sys.path:
                sys.path.insert(0, p)


_ensure_path()

import ml_dtypes  # noqa: E402
import concourse.bass as bass  # noqa: E402
import concourse.bacc as bacc  # noqa: E402
import concourse.tile as tile  # noqa: E402
import concourse.mybir as mybir  # noqa: E402
from concourse.bass_utils import run_bass_kernel_spmd  # noqa: E402

F32 = mybir.dt.float32
BF16 = mybir.dt.bfloat16
AL = mybir.AluOpType
AF = mybir.ActivationFunctionType
BF16NP = np.dtype(ml_dtypes.bfloat16)

MARGIN = 0.1
B, KP, KNN, N, M = 2, 256, 32, 8192, 8192
NCORES = 8
NSHARDS = NCORES // B          # 4 n-chunks per batch element
CHUNK = N // NSHARDS           # 2048
NJ = CHUNK // 512              # 4 psum banks per m-tile
MI = M // 128                  # 64 m-tiles
K21 = 21
BIGB = 1.0e30

_CACHE = {}


def _build():
    nc = bacc.Bacc("TRN2", target_bir_lowering=False, debug=False,
                   num_devices=NCORES)

    tA_d = nc.dram_tensor("tA", [K21, M], BF16, kind="ExternalInput")
    sA_d = nc.dram_tensor("sA", [K21, CHUNK], BF16, kind="ExternalInput")
    nt_d = nc.dram_tensor("nt", [128, MI], F32, kind="ExternalInput")
    kp_lhsT = nc.dram_tensor("kp_lhsT", [4, 2 * 3], F32, kind="ExternalInput")
    kp_rhs = nc.dram_tensor("kp_rhs", [4, 2 * KP], F32, kind="ExternalInput")
    tgt_kp = nc.dram_tensor("tgt_kp", [3, 2 * KP], F32, kind="ExternalInput")
    knn_src = nc.dram_tensor("knn_src", [128, 2 * 192], F32, kind="ExternalInput")
    knn_tgt = nc.dram_tensor("knn_tgt", [128, 2 * 192], F32, kind="ExternalInput")

    acc_o = nc.dram_tensor("acc", [128, CHUNK], BF16, kind="ExternalOutput")
    rowmin_o = nc.dram_tensor("rowmin", [128, MI], F32, kind="ExternalOutput")
    misc_o = nc.dram_tensor("misc", [128, 4], F32, kind="ExternalOutput")

    with tile.TileContext(nc) as tc:
        with (
            tc.tile_pool(name="const", bufs=1) as const,
            tc.tile_pool(name="qb", bufs=3) as qbp,
            tc.tile_pool(name="rt", bufs=2) as rtp,
        ):
            tA = const.tile([K21, M], BF16)
            sA = const.tile([K21, CHUNK], BF16)
            nt_sb = const.tile([128, MI], F32)
            acc = const.tile([128, CHUNK], BF16)
            rowbuf = const.tile([128, MI], F32)
            misc_sb = const.tile([128, 4], F32)

            nc.sync.dma_start(out=tA[:], in_=tA_d[:])
            nc.sync.dma_start(out=sA[:], in_=sA_d[:])
            nc.sync.dma_start(out=nt_sb[:], in_=nt_d[:])
            nc.gpsimd.memset(acc[:], BIGB)
            nc.gpsimd.memset(misc_sb[:], 0.0)

            with tc.tile_pool(name="psum_main", bufs=2, space="PSUM") as pm:
                for mi in range(MI):
                    pt = pm.tile([128, CHUNK], F32, tag="pt")
                    for nj in range(NJ):
                        nc.tensor.matmul(
                            pt[:, nj * 512:(nj + 1) * 512],
                            lhsT=tA[:, mi * 128:(mi + 1) * 128],
                            rhs=sA[:, nj * 512:(nj + 1) * 512],
                            start=True, stop=True,
                        )
                    qb = qbp.tile([128, CHUNK], BF16, tag="qb")
                    nc.scalar.add(out=qb[:], in_=pt[:], add=nt_sb[:, mi:mi + 1])
                    # colmin accumulate (min over m-tiles per n position)
                    nc.vector.tensor_tensor(acc[:], acc[:], qb[:], AL.min)
                    # rowmin tree (min over the n-chunk per m row)
                    r1 = rtp.tile([128, CHUNK // 2], BF16, tag="r1")
                    nc.vector.tensor_tensor(
                        r1[:], qb[:, :CHUNK // 2], qb[:, CHUNK // 2:], AL.min)
                    r2 = rtp.tile([128, CHUNK // 4], BF16, tag="r2")
                    nc.vector.tensor_tensor(
                        r2[:], r1[:, :CHUNK // 4], r1[:, CHUNK // 4:], AL.min)
                    nc.vector.tensor_reduce(
                        out=rowbuf[:, mi:mi + 1], in_=r2[:],
                        axis=mybir.AxisListType.X, op=AL.min)

            nc.sync.dma_start(out=acc_o[:], in_=acc[:])
            nc.sync.dma_start(out=rowmin_o[:], in_=rowbuf[:])

            # tiny keypoint / knn losses (both batch elements)
            with tc.tile_pool(name="psum_fin", bufs=2, space="PSUM") as pf:
                kp_l = const.tile([4, 2 * 3], F32)
                kp_r = const.tile([4, 2 * KP], F32)
                kp_t = const.tile([3, 2 * KP], F32)
                ks = const.tile([128, 2 * 192], F32)
                kt = const.tile([128, 2 * 192], F32)
                nc.sync.dma_start(out=kp_l[:], in_=kp_lhsT[:])
                nc.sync.dma_start(out=kp_r[:], in_=kp_rhs[:])
                nc.sync.dma_start(out=kp_t[:], in_=tgt_kp[:])
                nc.sync.dma_start(out=ks[:], in_=knn_src[:])
                nc.sync.dma_start(out=kt[:], in_=knn_tgt[:])
                for b in range(B):
                    pt2 = pf.tile([3, KP], F32, tag="kp")
                    nc.tensor.matmul(
                        pt2[:], lhsT=kp_l[:, b * 3:(b + 1) * 3],
                        rhs=kp_r[:, b * KP:(b + 1) * KP],
                        start=True, stop=True)
                    diff = rtp.tile([3, KP], F32, tag="kdiff")
                    nc.vector.tensor_sub(diff[:], pt2[:],
                                         kp_t[:, b * KP:(b + 1) * KP])
                    nc.vector.tensor_mul(diff[:], diff[:], diff[:])
                    nc.vector.tensor_reduce(
                        out=misc_sb[0:3, b:b + 1], in_=diff[:],
                        axis=mybir.AxisListType.X, op=AL.add)
                    diff2 = rtp.tile([128, 192], F32, tag="ndiff")
                    nc.vector.tensor_sub(diff2[:], ks[:, b * 192:(b + 1) * 192],
                                         kt[:, b * 192:(b + 1) * 192])
                    nc.vector.tensor_mul(diff2[:], diff2[:], diff2[:])
                    nc.vector.tensor_reduce(
                        out=misc_sb[:, 2 + b:3 + b], in_=diff2[:],
                        axis=mybir.AxisListType.X, op=AL.add)

            nc.sync.dma_start(out=misc_o[:], in_=misc_sb[:])

    nc.compile()
    return nc


def _get_nc():
    if "nc" not in _CACHE:
        _CACHE["nc"] = _build()
    return _CACHE["nc"]


def _split3(x):
    """Exact 3-way bf16 split of an fp32 array: x ~= h + m + l."""
    f = np.float32
    h = x.astype(BF16NP)
    r = x - h.astype(f)
    m = r.astype(BF16NP)
    r2 = r - m.astype(f)
    low = r2.astype(BF16NP)
    return h, m, low


# lhsT rows [th,th,tm,tm,th,tl]*3d + ones*3 ; rhs rows [sh,sm,sh,sm,sl,sh]*3d
# + the 3-way split of ||s||^2.  (pairing keeps the 6 dominant cross products)
_T_DEST = {0: [0, 3, 12], 1: [6, 9], 2: [15]}    # th, tm, tl row bases
_S_DEST = {0: [0, 6, 15], 1: [3, 9], 2: [12]}    # sh, sm, sl row bases


def _pack_rows(x, ns, width):
    """Build the [21, width] bf16 operand for one side.

    x: [3, width] fp32 (already scaled by -2 for the t side)
    ns: [width] fp32 squared-norm rows (s side) or None (t side -> ones)
    """
    out = np.zeros((K21, width), dtype=BF16NP)
    h, m, low = _split3(x)
    dest = _S_DEST if ns is not None else _T_DEST
    for lvl, w in enumerate((h, m, low)):
        for base in dest[lvl]:
            out[base:base + 3, :] = w
    if ns is None:
        out[18:21, :] = np.ones((3, width), dtype=BF16NP)
    else:
        nh, nm, nl = _split3(ns)
        out[18, :] = nh
        out[19, :] = nm
        out[20, :] = nl
    return out


def _prepare_in_maps(src_keypoints, tgt_keypoints, rotation_ab, translation_ab,
                     src_keypoints_knn, tgt_keypoints_knn, src_transformed, tgt):
    f = np.float32
    st = np.ascontiguousarray(np.asarray(src_transformed, dtype=f))
    tg = np.ascontiguousarray(np.asarray(tgt, dtype=f))
    skp = np.asarray(src_keypoints, dtype=f)
    tkp = np.asarray(tgt_keypoints, dtype=f)
    rot = np.asarray(rotation_ab, dtype=f)
    tra = np.asarray(translation_ab, dtype=f)
    sknn = np.asarray(src_keypoints_knn, dtype=f)
    tknn = np.asarray(tgt_keypoints_knn, dtype=f)

    kp_lhsT = np.zeros((4, 2 * 3), dtype=f)
    kp_rhs = np.zeros((4, 2 * KP), dtype=f)
    tgt_kp = np.zeros((3, 2 * KP), dtype=f)
    knn_src = np.zeros((128, 2 * 192), dtype=f)
    knn_tgt = np.zeros((128, 2 * 192), dtype=f)
    for b in range(B):
        kp_lhsT[0:3, b * 3:(b + 1) * 3] = rot[b].T
        kp_lhsT[3, b * 3:(b + 1) * 3] = tra[b]
        kp_rhs[0:3, b * KP:(b + 1) * KP] = skp[b]
        kp_rhs[3, b * KP:(b + 1) * KP] = 1.0
        tgt_kp[:, b * KP:(b + 1) * KP] = tkp[b]
        knn_src[:, b * 192:(b + 1) * 192] = sknn[b].reshape(128, 192)
        knn_tgt[:, b * 192:(b + 1) * 192] = tknn[b].reshape(128, 192)

    shared = {
        "kp_lhsT": kp_lhsT, "kp_rhs": kp_rhs,
        "tgt_kp": tgt_kp, "knn_src": knn_src, "knn_tgt": knn_tgt,
    }
    # per-batch t-side operand + ||t||^2 in the [p, g] layout (m = g*128 + p)
    tA_b, nt_b = [], []
    for b in range(B):
        t = tg[b]                                   # [3, M]
        tA_b.append(_pack_rows(-2.0 * t, None, M))
        nt = (t * t).sum(axis=0)                    # [M]
        nt_b.append(np.ascontiguousarray(nt.reshape(MI, 128).T))
    in_maps = []
    for ci in range(NCORES):
        b, j = divmod(ci, NSHARDS)
        s = np.ascontiguousarray(st[b, :, j * CHUNK:(j + 1) * CHUNK])
        ns = (s * s).sum(axis=0)
        mdict = dict(shared)
        mdict["tA"] = tA_b[b]
        mdict["nt"] = nt_b[b]
        mdict["sA"] = _pack_rows(s, ns, CHUNK)
        in_maps.append(mdict)
    return in_maps


def _huber(x, c):
    return np.where(x < c, 0.5 * x * x, c * x - 0.5 * c * c)


def _postprocess(results):
    c = np.float64(MARGIN)
    loss1 = np.float64(0.0)
    loss2 = np.float64(0.0)
    for b in range(B):
        rowmins = []
        for j in range(NSHARDS):
            r = results[b * NSHARDS + j]
            colmin = np.asarray(r["acc"]).astype(np.float64).min(axis=0)
            loss1 += _huber(colmin, c).sum()
            rowmins.append(np.asarray(r["rowmin"], dtype=np.float64).T.ravel())
        rm = np.minimum.reduce(rowmins)
        loss2 += _huber(rm, c).sum()
    gal = loss1 + loss2

    misc = np.asarray(results[0]["misc"], dtype=np.float64)
    kp_loss = (misc[0:3, 0].sum() + misc[0:3, 1].sum()) / B
    knn_loss = (misc[:, 2].sum() + misc[:, 3].sum()) / (B * KNN)
    ncl = knn_loss + kp_loss
    return np.float32(ncl), np.float32(gal)


def run_device(in_maps, **kw):
    nc = _get_nc()
    return run_bass_kernel_spmd(nc, in_maps, list(range(NCORES)), **kw)


def kernel(src_keypoints, tgt_keypoints, rotation_ab, translation_ab,
           src_keypoints_knn, tgt_keypoints_knn, k, src_transformed, tgt,
           **_unused):
    in_maps = _prepare_in_maps(src_keypoints, tgt_keypoints, rotation_ab,
                               translation_ab, src_keypoints_knn,
                               tgt_keypoints_knn, src_transformed, tgt)
    res = run_device(in_maps)
    return _postprocess(res.results)


# revision 10
# speedup vs baseline: 2.2807x; 1.0156x over previous
"""RIENet loss kernel (keypoint/KNN MSE + global-align Huber-min loss) on 8 trn2 cores.

Sharding: core ci -> (b = ci // 4, n-chunk j = ci % 4).  Each core holds the full
tgt[b] (M=8192 points) and a 2048-column chunk of src_transformed[b] (N axis).
  loss_1 (min over M per src point): per-core partial over the partition axis,
          finished on host (min over 128 partitions of the DMA'd acc tile).
  loss_2 (min over N per tgt point): per-core partial min over its chunk;
          host min-reduces the 4 chunks per batch element.

v5: all operand prep happens on the HOST (bf16 2-way splits, norms, row
layouts) so the device runs only the steady-state loop:
  PE     : Q[m-tile, :] = -2 t.s + ||s||^2 via K=11 bf16 matmuls (4 banks)
  ScalarE: Qb = bf16(Q + ||t_m||^2)   (activation Identity, per-partition bias)
  DVE    : acc  = min(acc, Qb)                     (TT bf16 SBUF, 2x mode)
           r1   = min(Qb[:, :1024], Qb[:, 1024:])  (TT 2x)
           r2oct[:, mi%8, :] = min(r1[:, :512], r1[:, 512:])  (TT 2x)
           every 8th tile: rowbuf[:, mi-7:mi+1] = reduce_min(r2oct) (batched)
acc ([128, 2048] bf16, min over m per (partition, n) position) is DMA'd out
raw; the cross-partition min + Huber + sums happen on host (tiny).
Tiny keypoint/KNN MSE losses run on-device at the end of every core.
"""

import os
import numpy as np


def _ensure_path():
    try:
        import concourse  # noqa: F401
    except ImportError:
        import sys
        for p in ("/opt/trn_rl_repo", "/root/.axon_site/_ro/trn_rl_repo"):
            if os.path.isdir(p) and p not in sys.path:
                sys.path.insert(0, p)


_ensure_path()

import ml_dtypes  # noqa: E402
import concourse.bass as bass  # noqa: E402
import concourse.bacc as bacc  # noqa: E402
import concourse.tile as tile  # noqa: E402
import concourse.mybir as mybir  # noqa: E402
from concourse.bass_utils import run_bass_kernel_spmd  # noqa: E402

F32 = mybir.dt.float32
BF16 = mybir.dt.bfloat16
AL = mybir.AluOpType
AF = mybir.ActivationFunctionType
BF16NP = np.dtype(ml_dtypes.bfloat16)

MARGIN = 0.1
B, KP, KNN, N, M = 2, 256, 32, 8192, 8192
NCORES = 8
NSHARDS = NCORES // B          # 4 n-chunks per batch element
CHUNK = N // NSHARDS           # 2048
NJ = CHUNK // 512              # 4 psum banks per m-tile
MI = M // 128                  # 64 m-tiles
K11 = 11
BIGB = 1.0e30

_CACHE = {}


def _build():
    nc = bacc.Bacc("TRN2", target_bir_lowering=False, debug=False,
                   num_devices=NCORES)

    tA_d = nc.dram_tensor("tA", [K11, M], BF16, kind="ExternalInput")
    sA_d = nc.dram_tensor("sA", [K11, CHUNK], BF16, kind="ExternalInput")
    nt_d = nc.dram_tensor("nt", [128, MI], F32, kind="ExternalInput")
    kp_lhsT = nc.dram_tensor("kp_lhsT", [4, 2 * 3], F32, kind="ExternalInput")
    kp_rhs = nc.dram_tensor("kp_rhs", [4, 2 * KP], F32, kind="ExternalInput")
    tgt_kp = nc.dram_tensor("tgt_kp", [3, 2 * KP], F32, kind="ExternalInput")
    knn_src = nc.dram_tensor("knn_src", [128, 2 * 192], F32, kind="ExternalInput")
    knn_tgt = nc.dram_tensor("knn_tgt", [128, 2 * 192], F32, kind="ExternalInput")

    acc_o = nc.dram_tensor("acc", [128, CHUNK], BF16, kind="ExternalOutput")
    rowmin_o = nc.dram_tensor("rowmin", [128, MI], F32, kind="ExternalOutput")
    misc_o = nc.dram_tensor("misc", [128, 4], F32, kind="ExternalOutput")

    with tile.TileContext(nc) as tc:
        with (
            tc.tile_pool(name="const", bufs=1) as const,
            tc.tile_pool(name="qb", bufs=3) as qbp,
            tc.tile_pool(name="rt", bufs=3) as rtp,
        ):
            tA = const.tile([K11, M], BF16)
            sA = const.tile([K11, CHUNK], BF16)
            nt_sb = const.tile([128, MI], F32)
            acc = const.tile([128, CHUNK], BF16)
            rowbuf = const.tile([128, MI], F32)
            misc_sb = const.tile([128, 4], F32)

            nc.sync.dma_start(out=tA[:], in_=tA_d[:])
            nc.sync.dma_start(out=sA[:], in_=sA_d[:])
            nc.sync.dma_start(out=nt_sb[:], in_=nt_d[:])
            nc.gpsimd.memset(acc[:], BIGB)
            nc.gpsimd.memset(misc_sb[:], 0.0)

            r2oct = const.tile([128, 8, CHUNK // 4], BF16)
            with tc.tile_pool(name="psum_main", bufs=2, space="PSUM") as pm:
                for mi in range(MI):
                    pt = pm.tile([128, CHUNK], F32, tag="pt")
                    for nj in range(NJ):
                        nc.tensor.matmul(
                            pt[:, nj * 512:(nj + 1) * 512],
                            lhsT=tA[:, mi * 128:(mi + 1) * 128],
                            rhs=sA[:, nj * 512:(nj + 1) * 512],
                            start=True, stop=True,
                        )
                    qb = qbp.tile([128, CHUNK], BF16, tag="qb")
                    nc.scalar.add(out=qb[:], in_=pt[:], add=nt_sb[:, mi:mi + 1])
                    # colmin accumulate (min over m-tiles per n position)
                    nc.vector.tensor_tensor(acc[:], acc[:], qb[:], AL.min)
                    # rowmin tree (min over the n-chunk per m row)
                    r1 = rtp.tile([128, CHUNK // 2], BF16, tag="r1")
                    nc.vector.tensor_tensor(
                        r1[:], qb[:, :CHUNK // 2], qb[:, CHUNK // 2:], AL.min)
                    nc.vector.tensor_tensor(
                        r2oct[:, mi % 8, :], r1[:, :CHUNK // 4],
                        r1[:, CHUNK // 4:], AL.min)
                    if mi % 8 == 7:
                        nc.vector.tensor_reduce(
                            out=rowbuf[:, mi - 7:mi + 1], in_=r2oct[:],
                            axis=mybir.AxisListType.X, op=AL.min)

            nc.sync.dma_start(out=acc_o[:], in_=acc[:])
            nc.sync.dma_start(out=rowmin_o[:], in_=rowbuf[:])

            # tiny keypoint / knn losses (both batch elements)
            with tc.tile_pool(name="psum_fin", bufs=2, space="PSUM") as pf:
                kp_l = const.tile([4, 2 * 3], F32)
                kp_r = const.tile([4, 2 * KP], F32)
                kp_t = const.tile([3, 2 * KP], F32)
                ks = const.tile([128, 2 * 192], F32)
                kt = const.tile([128, 2 * 192], F32)
                nc.sync.dma_start(out=kp_l[:], in_=kp_lhsT[:])
                nc.sync.dma_start(out=kp_r[:], in_=kp_rhs[:])
                nc.sync.dma_start(out=kp_t[:], in_=tgt_kp[:])
                nc.sync.dma_start(out=ks[:], in_=knn_src[:])
                nc.sync.dma_start(out=kt[:], in_=knn_tgt[:])
                for b in range(B):
                    pt2 = pf.tile([3, KP], F32, tag="kp")
                    nc.tensor.matmul(
                        pt2[:], lhsT=kp_l[:, b * 3:(b + 1) * 3],
                        rhs=kp_r[:, b * KP:(b + 1) * KP],
                        start=True, stop=True)
                    diff = rtp.tile([3, KP], F32, tag="kdiff")
                    nc.vector.tensor_sub(diff[:], pt2[:],
                                         kp_t[:, b * KP:(b + 1) * KP])
                    nc.vector.tensor_mul(diff[:], diff[:], diff[:])
                    nc.vector.tensor_reduce(
                        out=misc_sb[0:3, b:b + 1], in_=diff[:],
                        axis=mybir.AxisListType.X, op=AL.add)
                    diff2 = rtp.tile([128, 192], F32, tag="ndiff")
                    nc.vector.tensor_sub(diff2[:], ks[:, b * 192:(b + 1) * 192],
                                         kt[:, b * 192:(b + 1) * 192])
                    nc.vector.tensor_mul(diff2[:], diff2[:], diff2[:])
                    nc.vector.tensor_reduce(
                        out=misc_sb[:, 2 + b:3 + b], in_=diff2[:],
                        axis=mybir.AxisListType.X, op=AL.add)

            nc.sync.dma_start(out=misc_o[:], in_=misc_sb[:])

    nc.compile()
    return nc


def _get_nc():
    if "nc" not in _CACHE:
        _CACHE["nc"] = _build()
    return _CACHE["nc"]


def _split2(x):
    """Exact 2-way bf16 split of an fp32 array: x ~= h + m."""
    f = np.float32
    h = x.astype(BF16NP)
    m = (x - h.astype(f)).astype(BF16NP)
    return h, m


# lhsT rows [th,th,tm]*3d + ones*2 ; rhs rows [sh,sm,sh]*3d + 2-way split of
# ||s||^2.  (keeps the 3 dominant cross products; tm*sm term ~2^-18 dropped)
_T_DEST = {0: [0, 3], 1: [6]}    # th, tm row bases
_S_DEST = {0: [0, 6], 1: [3]}    # sh, sm row bases


def _pack_rows(x, ns, width):
    """Build the [11, width] bf16 operand for one side.

    x: [3, width] fp32 (already scaled by -2 for the t side)
    ns: [width] fp32 squared-norm rows (s side) or None (t side -> ones)
    """
    out = np.zeros((K11, width), dtype=BF16NP)
    h, m = _split2(x)
    dest = _S_DEST if ns is not None else _T_DEST
    for lvl, w in enumerate((h, m)):
        for base in dest[lvl]:
            out[base:base + 3, :] = w
    if ns is None:
        out[9:11, :] = np.ones((2, width), dtype=BF16NP)
    else:
        nh, nm = _split2(ns)
        out[9, :] = nh
        out[10, :] = nm
    return out


def _prepare_in_maps(src_keypoints, tgt_keypoints, rotation_ab, translation_ab,
                     src_keypoints_knn, tgt_keypoints_knn, src_transformed, tgt):
    f = np.float32
    st = np.ascontiguousarray(np.asarray(src_transformed, dtype=f))
    tg = np.ascontiguousarray(np.asarray(tgt, dtype=f))
    skp = np.asarray(src_keypoints, dtype=f)
    tkp = np.asarray(tgt_keypoints, dtype=f)
    rot = np.asarray(rotation_ab, dtype=f)
    tra = np.asarray(translation_ab, dtype=f)
    sknn = np.asarray(src_keypoints_knn, dtype=f)
    tknn = np.asarray(tgt_keypoints_knn, dtype=f)

    kp_lhsT = np.zeros((4, 2 * 3), dtype=f)
    kp_rhs = np.zeros((4, 2 * KP), dtype=f)
    tgt_kp = np.zeros((3, 2 * KP), dtype=f)
    knn_src = np.zeros((128, 2 * 192), dtype=f)
    knn_tgt = np.zeros((128, 2 * 192), dtype=f)
    for b in range(B):
        kp_lhsT[0:3, b * 3:(b + 1) * 3] = rot[b].T
        kp_lhsT[3, b * 3:(b + 1) * 3] = tra[b]
        kp_rhs[0:3, b * KP:(b + 1) * KP] = skp[b]
        kp_rhs[3, b * KP:(b + 1) * KP] = 1.0
        tgt_kp[:, b * KP:(b + 1) * KP] = tkp[b]
        knn_src[:, b * 192:(b + 1) * 192] = sknn[b].reshape(128, 192)
        knn_tgt[:, b * 192:(b + 1) * 192] = tknn[b].reshape(128, 192)

    shared = {
        "kp_lhsT": kp_lhsT, "kp_rhs": kp_rhs,
        "tgt_kp": tgt_kp, "knn_src": knn_src, "knn_tgt": knn_tgt,
    }
    # per-batch t-side operand + ||t||^2 in the [p, g] layout (m = g*128 + p)
    tA_b, nt_b = [], []
    for b in range(B):
        t = tg[b]                                   # [3, M]
        tA_b.append(_pack_rows(-2.0 * t, None, M))
        nt = (t * t).sum(axis=0)                    # [M]
        nt_b.append(np.ascontiguousarray(nt.reshape(MI, 128).T))
    in_maps = []
    for ci in range(NCORES):
        b, j = divmod(ci, NSHARDS)
        s = np.ascontiguousarray(st[b, :, j * CHUNK:(j + 1) * CHUNK])
        ns = (s * s).sum(axis=0)
        mdict = dict(shared)
        mdict["tA"] = tA_b[b]
        mdict["nt"] = nt_b[b]
        mdict["sA"] = _pack_rows(s, ns, CHUNK)
        in_maps.append(mdict)
    return in_maps


def _huber(x, c):
    return np.where(x < c, 0.5 * x * x, c * x - 0.5 * c * c)


def _postprocess(results):
    c = np.float64(MARGIN)
    loss1 = np.float64(0.0)
    loss2 = np.float64(0.0)
    for b in range(B):
        rowmins = []
        for j in range(NSHARDS):
            r = results[b * NSHARDS + j]
            colmin = np.asarray(r["acc"]).astype(np.float64).min(axis=0)
            loss1 += _huber(colmin, c).sum()
            rowmins.append(np.asarray(r["rowmin"], dtype=np.float64).T.ravel())
        rm = np.minimum.reduce(rowmins)
        loss2 += _huber(rm, c).sum()
    gal = loss1 + loss2

    misc = np.asarray(results[0]["misc"], dtype=np.float64)
    kp_loss = (misc[0:3, 0].sum() + misc[0:3, 1].sum()) / B
    knn_loss = (misc[:, 2].sum() + misc[:, 3].sum()) / (B * KNN)
    ncl = knn_loss + kp_loss
    return np.float32(ncl), np.float32(gal)


def run_device(in_maps, **kw):
    nc = _get_nc()
    return run_bass_kernel_spmd(nc, in_maps, list(range(NCORES)), **kw)


def kernel(src_keypoints, tgt_keypoints, rotation_ab, translation_ab,
           src_keypoints_knn, tgt_keypoints_knn, k, src_transformed, tgt,
           **_unused):
    in_maps = _prepare_in_maps(src_keypoints, tgt_keypoints, rotation_ab,
                               translation_ab, src_keypoints_knn,
                               tgt_keypoints_knn, src_transformed, tgt)
    res = run_device(in_maps)
    return _postprocess(res.results)


# revision 11
# speedup vs baseline: 2.9479x; 1.2926x over previous
"""RIENet loss kernel (keypoint/KNN MSE + global-align Huber-min loss) on 8 trn2 cores.

Sharding: core ci -> (b = ci // 4, n-chunk j = ci % 4).  Each core holds the full
tgt[b] (M=8192 points) and a 2048-column chunk of src_transformed[b] (N axis).
  loss_1 (min over M per src point): per-core partial over the partition axis,
          finished on host (min over 128 partitions of the DMA'd acc tile).
  loss_2 (min over N per tgt point): per-core partial min over its chunk;
          host min-reduces the 4 chunks per batch element.

v5: all operand prep happens on the HOST (bf16 2-way splits, norms, row
layouts) so the device runs only the steady-state loop:
  PE     : Q[m-tile, :] = -2 t.s + ||s||^2 via K=11 bf16 matmuls (4 banks)
  ScalarE: Qb = bf16(Q + ||t_m||^2)   (activation Identity, per-partition bias)
  DVE    : acc  = min(acc, Qb)                     (TT bf16 SBUF, 2x mode)
           r1   = min(Qb[:, :1024], Qb[:, 1024:])  (TT 2x)
  DMA    : r1 streams out per tile ([128, 64, 1024] bf16 total)
acc ([128, 2048] bf16, min over m per (partition, n) position) is DMA'd out
raw; the rowmin tail (1024-way min per row), the cross-partition colmin, and
the Huber + sums happen on host (u16 bit-trick min, ~10 ms).
Tiny keypoint/KNN MSE losses run on-device at the end of every core.
"""

import os
import numpy as np


def _ensure_path():
    try:
        import concourse  # noqa: F401
    except ImportError:
        import sys
        for p in ("/opt/trn_rl_repo", "/root/.axon_site/_ro/trn_rl_repo"):
            if os.path.isdir(p) and p not in sys.path:
                sys.path.insert(0, p)


_ensure_path()

import ml_dtypes  # noqa: E402
import concourse.bass as bass  # noqa: E402
import concourse.bacc as bacc  # noqa: E402
import concourse.tile as tile  # noqa: E402
import concourse.mybir as mybir  # noqa: E402
from concourse.bass_utils import run_bass_kernel_spmd  # noqa: E402

F32 = mybir.dt.float32
BF16 = mybir.dt.bfloat16
AL = mybir.AluOpType
AF = mybir.ActivationFunctionType
BF16NP = np.dtype(ml_dtypes.bfloat16)

MARGIN = 0.1
B, KP, KNN, N, M = 2, 256, 32, 8192, 8192
NCORES = 8
NSHARDS = NCORES // B          # 4 n-chunks per batch element
CHUNK = N // NSHARDS           # 2048
NJ = CHUNK // 512              # 4 psum banks per m-tile
MI = M // 128                  # 64 m-tiles
K11 = 11
BIGB = 1.0e30

_CACHE = {}


def _build():
    nc = bacc.Bacc("TRN2", target_bir_lowering=False, debug=False,
                   num_devices=NCORES)

    tA_d = nc.dram_tensor("tA", [K11, M], BF16, kind="ExternalInput")
    sA_d = nc.dram_tensor("sA", [K11, CHUNK], BF16, kind="ExternalInput")
    nt_d = nc.dram_tensor("nt", [128, MI], F32, kind="ExternalInput")
    kp_lhsT = nc.dram_tensor("kp_lhsT", [4, 2 * 3], F32, kind="ExternalInput")
    kp_rhs = nc.dram_tensor("kp_rhs", [4, 2 * KP], F32, kind="ExternalInput")
    tgt_kp = nc.dram_tensor("tgt_kp", [3, 2 * KP], F32, kind="ExternalInput")
    knn_src = nc.dram_tensor("knn_src", [128, 2 * 192], F32, kind="ExternalInput")
    knn_tgt = nc.dram_tensor("knn_tgt", [128, 2 * 192], F32, kind="ExternalInput")

    acc_o = nc.dram_tensor("acc", [128, CHUNK], BF16, kind="ExternalOutput")
    r1_o = nc.dram_tensor("r1o", [128, MI, CHUNK // 2], BF16,
                          kind="ExternalOutput")
    misc_o = nc.dram_tensor("misc", [128, 4], F32, kind="ExternalOutput")

    with tile.TileContext(nc) as tc:
        with (
            tc.tile_pool(name="const", bufs=1) as const,
            tc.tile_pool(name="qb", bufs=3) as qbp,
            tc.tile_pool(name="rt", bufs=3) as rtp,
        ):
            tA = const.tile([K11, M], BF16)
            sA = const.tile([K11, CHUNK], BF16)
            nt_sb = const.tile([128, MI], F32)
            acc = const.tile([128, CHUNK], BF16)
            misc_sb = const.tile([128, 4], F32)

            nc.sync.dma_start(out=tA[:], in_=tA_d[:])
            nc.sync.dma_start(out=sA[:], in_=sA_d[:])
            nc.sync.dma_start(out=nt_sb[:], in_=nt_d[:])
            nc.gpsimd.memset(acc[:], BIGB)
            nc.gpsimd.memset(misc_sb[:], 0.0)

            with tc.tile_pool(name="psum_main", bufs=2, space="PSUM") as pm:
                for mi in range(MI):
                    pt = pm.tile([128, CHUNK], F32, tag="pt")
                    for nj in range(NJ):
                        nc.tensor.matmul(
                            pt[:, nj * 512:(nj + 1) * 512],
                            lhsT=tA[:, mi * 128:(mi + 1) * 128],
                            rhs=sA[:, nj * 512:(nj + 1) * 512],
                            start=True, stop=True,
                        )
                    qb = qbp.tile([128, CHUNK], BF16, tag="qb")
                    nc.scalar.add(out=qb[:], in_=pt[:], add=nt_sb[:, mi:mi + 1])
                    # colmin accumulate (min over m-tiles per n position)
                    nc.vector.tensor_tensor(acc[:], acc[:], qb[:], AL.min)
                    # rowmin tree (min over the n-chunk per m row)
                    r1 = rtp.tile([128, CHUNK // 2], BF16, tag="r1")
                    nc.vector.tensor_tensor(
                        r1[:], qb[:, :CHUNK // 2], qb[:, CHUNK // 2:], AL.min)
                    nc.sync.dma_start(out=r1_o[:, mi, :], in_=r1[:])

            nc.sync.dma_start(out=acc_o[:], in_=acc[:])

            # tiny keypoint / knn losses (both batch elements)
            with tc.tile_pool(name="psum_fin", bufs=2, space="PSUM") as pf:
                kp_l = const.tile([4, 2 * 3], F32)
                kp_r = const.tile([4, 2 * KP], F32)
                kp_t = const.tile([3, 2 * KP], F32)
                ks = const.tile([128, 2 * 192], F32)
                kt = const.tile([128, 2 * 192], F32)
                nc.sync.dma_start(out=kp_l[:], in_=kp_lhsT[:])
                nc.sync.dma_start(out=kp_r[:], in_=kp_rhs[:])
                nc.sync.dma_start(out=kp_t[:], in_=tgt_kp[:])
                nc.sync.dma_start(out=ks[:], in_=knn_src[:])
                nc.sync.dma_start(out=kt[:], in_=knn_tgt[:])
                for b in range(B):
                    pt2 = pf.tile([3, KP], F32, tag="kp")
                    nc.tensor.matmul(
                        pt2[:], lhsT=kp_l[:, b * 3:(b + 1) * 3],
                        rhs=kp_r[:, b * KP:(b + 1) * KP],
                        start=True, stop=True)
                    diff = rtp.tile([3, KP], F32, tag="kdiff")
                    nc.vector.tensor_sub(diff[:], pt2[:],
                                         kp_t[:, b * KP:(b + 1) * KP])
                    nc.vector.tensor_mul(diff[:], diff[:], diff[:])
                    nc.vector.tensor_reduce(
                        out=misc_sb[0:3, b:b + 1], in_=diff[:],
                        axis=mybir.AxisListType.X, op=AL.add)
                    diff2 = rtp.tile([128, 192], F32, tag="ndiff")
                    nc.vector.tensor_sub(diff2[:], ks[:, b * 192:(b + 1) * 192],
                                         kt[:, b * 192:(b + 1) * 192])
                    nc.vector.tensor_mul(diff2[:], diff2[:], diff2[:])
                    nc.vector.tensor_reduce(
                        out=misc_sb[:, 2 + b:3 + b], in_=diff2[:],
                        axis=mybir.AxisListType.X, op=AL.add)

            nc.sync.dma_start(out=misc_o[:], in_=misc_sb[:])

    nc.compile()
    return nc


def _get_nc():
    if "nc" not in _CACHE:
        _CACHE["nc"] = _build()
    return _CACHE["nc"]


def _split2(x):
    """Exact 2-way bf16 split of an fp32 array: x ~= h + m."""
    f = np.float32
    h = x.astype(BF16NP)
    m = (x - h.astype(f)).astype(BF16NP)
    return h, m


# lhsT rows [th,th,tm]*3d + ones*2 ; rhs rows [sh,sm,sh]*3d + 2-way split of
# ||s||^2.  (keeps the 3 dominant cross products; tm*sm term ~2^-18 dropped)
_T_DEST = {0: [0, 3], 1: [6]}    # th, tm row bases
_S_DEST = {0: [0, 6], 1: [3]}    # sh, sm row bases


def _pack_rows(x, ns, width):
    """Build the [11, width] bf16 operand for one side.

    x: [3, width] fp32 (already scaled by -2 for the t side)
    ns: [width] fp32 squared-norm rows (s side) or None (t side -> ones)
    """
    out = np.zeros((K11, width), dtype=BF16NP)
    h, m = _split2(x)
    dest = _S_DEST if ns is not None else _T_DEST
    for lvl, w in enumerate((h, m)):
        for base in dest[lvl]:
            out[base:base + 3, :] = w
    if ns is None:
        out[9:11, :] = np.ones((2, width), dtype=BF16NP)
    else:
        nh, nm = _split2(ns)
        out[9, :] = nh
        out[10, :] = nm
    return out


def _prepare_in_maps(src_keypoints, tgt_keypoints, rotation_ab, translation_ab,
                     src_keypoints_knn, tgt_keypoints_knn, src_transformed, tgt):
    f = np.float32
    st = np.ascontiguousarray(np.asarray(src_transformed, dtype=f))
    tg = np.ascontiguousarray(np.asarray(tgt, dtype=f))
    skp = np.asarray(src_keypoints, dtype=f)
    tkp = np.asarray(tgt_keypoints, dtype=f)
    rot = np.asarray(rotation_ab, dtype=f)
    tra = np.asarray(translation_ab, dtype=f)
    sknn = np.asarray(src_keypoints_knn, dtype=f)
    tknn = np.asarray(tgt_keypoints_knn, dtype=f)

    kp_lhsT = np.zeros((4, 2 * 3), dtype=f)
    kp_rhs = np.zeros((4, 2 * KP), dtype=f)
    tgt_kp = np.zeros((3, 2 * KP), dtype=f)
    knn_src = np.zeros((128, 2 * 192), dtype=f)
    knn_tgt = np.zeros((128, 2 * 192), dtype=f)
    for b in range(B):
        kp_lhsT[0:3, b * 3:(b + 1) * 3] = rot[b].T
        kp_lhsT[3, b * 3:(b + 1) * 3] = tra[b]
        kp_rhs[0:3, b * KP:(b + 1) * KP] = skp[b]
        kp_rhs[3, b * KP:(b + 1) * KP] = 1.0
        tgt_kp[:, b * KP:(b + 1) * KP] = tkp[b]
        knn_src[:, b * 192:(b + 1) * 192] = sknn[b].reshape(128, 192)
        knn_tgt[:, b * 192:(b + 1) * 192] = tknn[b].reshape(128, 192)

    shared = {
        "kp_lhsT": kp_lhsT, "kp_rhs": kp_rhs,
        "tgt_kp": tgt_kp, "knn_src": knn_src, "knn_tgt": knn_tgt,
    }
    # per-batch t-side operand + ||t||^2 in the [p, g] layout (m = g*128 + p)
    tA_b, nt_b = [], []
    for b in range(B):
        t = tg[b]                                   # [3, M]
        tA_b.append(_pack_rows(-2.0 * t, None, M))
        nt = (t * t).sum(axis=0)                    # [M]
        nt_b.append(np.ascontiguousarray(nt.reshape(MI, 128).T))
    in_maps = []
    for ci in range(NCORES):
        b, j = divmod(ci, NSHARDS)
        s = np.ascontiguousarray(st[b, :, j * CHUNK:(j + 1) * CHUNK])
        ns = (s * s).sum(axis=0)
        mdict = dict(shared)
        mdict["tA"] = tA_b[b]
        mdict["nt"] = nt_b[b]
        mdict["sA"] = _pack_rows(s, ns, CHUNK)
        in_maps.append(mdict)
    return in_maps


def _huber(x, c):
    return np.where(x < c, 0.5 * x * x, c * x - 0.5 * c * c)


def _rowmin_host(r1o):
    """Per-row min over the last axis of a [128, MI, CHUNK//2] bf16 array.

    Uses the uint16 bit-pattern trick (valid for non-negative bf16); falls
    back to fp32 if any negative value is present (only possible within
    ~1e-4 of zero, where Huber is ~0 either way).
    """
    u = np.asarray(r1o).view(np.uint16)
    if (u & 0x8000).any():
        return np.asarray(r1o).astype(np.float32).min(axis=-1)
    return u.min(axis=-1).view(BF16NP).astype(np.float32)


def _postprocess(results):
    c = np.float64(MARGIN)
    loss1 = np.float64(0.0)
    loss2 = np.float64(0.0)
    for b in range(B):
        rowmins = []
        for j in range(NSHARDS):
            r = results[b * NSHARDS + j]
            colmin = np.asarray(r["acc"]).astype(np.float64).min(axis=0)
            loss1 += _huber(colmin, c).sum()
            rm_core = _rowmin_host(r["r1o"])          # [128, MI]
            rowmins.append(rm_core.astype(np.float64).T.ravel())
        rm = np.minimum.reduce(rowmins)
        loss2 += _huber(rm, c).sum()
    gal = loss1 + loss2

    misc = np.asarray(results[0]["misc"], dtype=np.float64)
    kp_loss = (misc[0:3, 0].sum() + misc[0:3, 1].sum()) / B
    knn_loss = (misc[:, 2].sum() + misc[:, 3].sum()) / (B * KNN)
    ncl = knn_loss + kp_loss
    return np.float32(ncl), np.float32(gal)


def run_device(in_maps, **kw):
    nc = _get_nc()
    return run_bass_kernel_spmd(nc, in_maps, list(range(NCORES)), **kw)


def kernel(src_keypoints, tgt_keypoints, rotation_ab, translation_ab,
           src_keypoints_knn, tgt_keypoints_knn, k, src_transformed, tgt,
           **_unused):
    in_maps = _prepare_in_maps(src_keypoints, tgt_keypoints, rotation_ab,
                               translation_ab, src_keypoints_knn,
                               tgt_keypoints_knn, src_transformed, tgt)
    res = run_device(in_maps)
    return _postprocess(res.results)
